# revision 1
# baseline (speedup 1.0000x reference)
"""GAT (2-layer, 4-head) distributed Bass kernel for Trainium2, 8 NeuronCores.

Strategy (1D node partition, dst-owner edge routing):
  - Core c owns nodes [c*NLOC, (c+1)*NLOC), padded to NLOCP = T*128.
  - Per layer: each core computes feat/el/er for its own nodes via PE matmuls
    (feat = x @ W, el = x @ (W@al), er = x @ (W@ar)), writes a [NLOCP, 192]
    "featel" table ([feat(128) | el(4) | pad]) plus a [NLOCP, 64] er table
    ([er(4) | pad]) to DRAM, and AllGathers featel across the 8 cores.
  - Edges are grouped by destination owner, then by 128-row destination tile.
    Within a tile, edges are split by source-table half (A: padded global src
    id < HALF, B: >= HALF) so dma_gather's int16 indices stay in range; each
    half is padded to whole 128-edge chunks. Chunk counts per (tile, half)
    are maxed across cores so the SPMD IR is identical on all 8 cores.
  - Per dst tile: batched dma_gathers (<=1024 rows per call) fetch the
    768-byte featel rows by src and the 256-byte er rows by dst. Per chunk
    of 128 edges:
      * logits = el[src] + er[dst]            (DVE add, [128,4])
      * s = exp(leakyrelu(logits))            (DVE stt + ACT exp)
      * one-hot O[e, r] = (dstrow[e] == r)    (DVE tensor_scalar is_equal)
      * featw = feat * s (head-broadcast); s into 4 denominator columns
      * PSUM accumulate: agg[r, :] += O.T @ featw  (numerator | denominator)
    Pad slots carry dstrow = -1 so their one-hot column is zero.
  - Per dst tile epilogue: rst = num/max(den,1e-9) + residual (+bias);
    layer 1 applies ELU and transposes h for layer 2's node matmuls;
    layer 2 takes the head-mean and stores the output.

Single-pass softmax: alpha = exp(e)/sum(exp(e)) == reference's
exp(e-emax)/sum(exp(e-emax)); logits are O(1) so no overflow.
"""

import numpy as np

# ---- problem constants (hardcoded; kernel.py must be self-contained) ----
N = 50000
E = 800000
P = 8
IN = 128
HID = 32
H = 4
F = H * HID          # 128, same for both layers
OUTD = 32
NEG = 0.2
TILE = 128

NLOC = N // P        # 6250
T = (NLOC + TILE - 1) // TILE          # 49
NLOCP = T * TILE     # 6272

ROWW = 192           # featel table row width (768 B)
ERW = 64             # er table row width (256 B)
import os as _os
BF16_AGG = _os.environ.get("GAT_BF16_AGG", "") == "1"
# bf16 one-hot + weighted features for the PE aggregation is ~10% faster but
# costs ~2e-3 relative error; default stays fp32-exact (~2e-6).


def _wrap16(idx):
    """[n] index list -> [128, n//16] int16, wrapped in 16 partitions and
    replicated across the 8 Q7 cores (dma_gather layout)."""
    a = np.asarray(idx).reshape(-1, 16).T
    return np.tile(a, (8, 1)).astype(np.int16)


# ----------------------------------------------------------------------------
# Host-side preprocessing
# ----------------------------------------------------------------------------

def prep_edges(src, dst, n=N, p=P):
    """Group edges by (dst owner, dst tile, src-half), pad each (core,tile,
    half) to common chunk counts KA_t/KB_t, and emit per-core index arrays.

    Returns (KAs, KBs, per_core): per_core[c] has
      gA   int16 [128, 8*sumKA]  wrapped featel-gather idxs, A half
      gB   int16 [128, 8*sumKB]  wrapped featel-gather idxs, B half (rebased)
      ger  int16 [128, 8*sumK]   wrapped er-gather idxs (dst local row)
      dstrow f32 [128, sumK]     dst row within tile per edge slot (-1 = pad)
    """
    nloc = n // p
    t_tiles = (nloc + TILE - 1) // TILE
    nlocp = t_tiles * TILE
    half = (p // 2) * nlocp

    owner = dst // nloc
    loc = dst - owner * nloc
    tl = loc // TILE
    row = loc - tl * TILE

    sowner = src // nloc
    pgid = sowner * nlocp + (src - sowner * nloc)
    hb = (pgid >= half).astype(np.int64)          # 0 = A, 1 = B

    order = np.lexsort((hb, tl, owner))
    owner_s = owner[order]
    tl_s = tl[order]
    hb_s = hb[order]
    row_s = row[order].astype(np.float32)
    loc_s = loc[order].astype(np.int32)
    pgid_s = pgid[order].astype(np.int64)

    counts = np.zeros((p, t_tiles, 2), dtype=np.int64)
    np.add.at(counts, (owner_s, tl_s, hb_s), 1)
    KAs = (-(-counts[:, :, 0] // TILE)).max(axis=0)
    KBs = (-(-counts[:, :, 1] // TILE)).max(axis=0)
    KAs = np.maximum(KAs, (KAs + KBs) == 0)       # ensure >=1 chunk per tile
    Ks = KAs + KBs
    off = np.concatenate([[0], np.cumsum(Ks)]).astype(int)
    offA = np.concatenate([[0], np.cumsum(KAs)]).astype(int)
    offB = np.concatenate([[0], np.cumsum(KBs)]).astype(int)
    sumK, sumKA, sumKB = int(off[-1]), int(offA[-1]), int(offB[-1])

    grp = (owner_s * t_tiles + tl_s) * 2 + hb_s
    gcnt = np.bincount(grp, minlength=p * t_tiles * 2)
    gstart = np.concatenate([[0], np.cumsum(gcnt)])
    within = np.arange(len(src)) - gstart[grp]
    k = within // TILE
    prt = within - k * TILE
    # chunk column in the full per-tile layout (A chunks first, then B)
    col = off[tl_s] + np.where(hb_s == 0, k, KAs[tl_s] + k)

    per_core = []
    for c in range(p):
        m = owner_s == c
        dstrow = np.full((TILE, sumK), -1.0, dtype=np.float32)
        dstrow[prt[m], col[m]] = row_s[m]
        erl = np.zeros((TILE, sumK), dtype=np.int64)
        erl[prt[m], col[m]] = loc_s[m]
        gfull = np.zeros((TILE, sumK), dtype=np.int64)
        gfull[prt[m], col[m]] = pgid_s[m]

        # flatten chunk cols into wrapped idx streams
        gA = np.zeros((TILE, 8 * sumKA), dtype=np.int16)
        gB = np.zeros((TILE, 8 * sumKB), dtype=np.int16)
        ger = np.zeros((TILE, 8 * sumK), dtype=np.int16)
        for t in range(t_tiles):
            ka, kb = int(KAs[t]), int(KBs[t])
            o, oa, ob = off[t], offA[t], offB[t]
            if ka:
                ia = gfull[:, o:o + ka].T.reshape(-1)          # i = k*128+p
                gA[:, 8 * oa:8 * (oa + ka)] = _wrap16(ia)
            if kb:
                ib = gfull[:, o + ka:o + ka + kb].T.reshape(-1) - half
                ib[ib < 0] = 0                                 # pad slots
                gB[:, 8 * ob:8 * (ob + kb)] = _wrap16(ib)
            ie = erl[:, o:o + ka + kb].T.reshape(-1)
            ger[:, 8 * o:8 * (o + ka + kb)] = _wrap16(ie)
        per_core.append(dict(gA=gA, gB=gB, ger=ger, dstrow=dstrow))
    return [int(x) for x in KAs], [int(x) for x in KBs], per_core


def prep_weights(W, al, ar):
    """[W | W@al per head | W@ar per head] -> [in, F+2H] float32."""
    Wr = W.reshape(W.shape[0], H, -1)
    wal = np.einsum('ihd,hd->ih', Wr, al)
    war = np.einsum('ihd,hd->ih', Wr, ar)
    return np.concatenate([W, wal, war], axis=1).astype(np.float32)


def prep_node_inputs(x, b1, n=N, p=P):
    """Per-core xT ([IN, NLOCP], lhsT layout) and xb ([128, T*IN],
    tile-row-major residual layout, bias prefolded)."""
    nloc = n // p
    t_tiles = (nloc + TILE - 1) // TILE
    nlocp = t_tiles * TILE
    outs = []
    for c in range(p):
        xl = np.zeros((nlocp, x.shape[1]), dtype=np.float32)
        xl[:nloc] = x[c * nloc:(c + 1) * nloc]
        xT = np.ascontiguousarray(xl.T)
        xb = (xl + b1[None, :]).reshape(t_tiles, TILE, -1).transpose(1, 0, 2)
        xb = np.ascontiguousarray(xb.reshape(TILE, -1))
        outs.append((xT, xb))
    return outs


# ----------------------------------------------------------------------------
# Bass IR builder
# ----------------------------------------------------------------------------

def build_gat(KAs, KBs, n=N, p=P, in_dim=IN):
    import concourse.bass as bass
    import concourse.bacc as bacc
    import concourse.mybir as mybir
    import concourse.tile as tile

    f32 = mybir.dt.float32
    i16 = mybir.dt.int16
    AF = mybir.ActivationFunctionType
    ALU = mybir.AluOpType

    nloc = n // p
    t_tiles = (nloc + TILE - 1) // TILE
    nlocp = t_tiles * TILE
    half = (p // 2) * nlocp
    KAs = list(KAs)
    KBs = list(KBs)
    Ks = [a + b for a, b in zip(KAs, KBs)]
    off = np.concatenate([[0], np.cumsum(Ks)]).astype(int)
    offA = np.concatenate([[0], np.cumsum(KAs)]).astype(int)
    offB = np.concatenate([[0], np.cumsum(KBs)]).astype(int)
    sumK, sumKA, sumKB = int(off[-1]), int(offA[-1]), int(offB[-1])
    Kmax = max(Ks)
    rg = [list(range(p))]

    nc = bacc.Bacc("TRN2", target_bir_lowering=False)

    # ---- I/O ----
    xT_in = nc.dram_tensor("xT", [in_dim, nlocp], f32, kind="ExternalInput")
    xb_in = nc.dram_tensor("xb", [TILE, t_tiles * in_dim], f32, kind="ExternalInput")
    W1_in = nc.dram_tensor("Wcat1", [in_dim, F + 2 * H], f32, kind="ExternalInput")
    W2_in = nc.dram_tensor("Wcat2", [F, F + 2 * H], f32, kind="ExternalInput")
    b2r_in = nc.dram_tensor("b2r", [TILE, F], f32, kind="ExternalInput")
    iota_in = nc.dram_tensor("iota", [TILE, TILE], f32, kind="ExternalInput")
    ident_in = nc.dram_tensor("ident", [TILE, TILE], f32, kind="ExternalInput")
    gA_in = nc.dram_tensor("gA", [TILE, 8 * sumKA], i16, kind="ExternalInput")
    gB_in = nc.dram_tensor("gB", [TILE, max(8 * sumKB, 16)], i16, kind="ExternalInput")
    ger_in = nc.dram_tensor("ger", [TILE, 8 * sumK], i16, kind="ExternalInput")
    dstrow_in = nc.dram_tensor("dstrow", [TILE, sumK], f32, kind="ExternalInput")
    out_ext = nc.dram_tensor("out", [nlocp, OUTD], f32, kind="ExternalOutput")

    # ---- internal DRAM ----
    fel_loc = [nc.dram_tensor(f"fel_loc{i}", [nlocp, ROWW], f32) for i in (1, 2)]
    fel_full = [nc.dram_tensor(f"fel_full{i}", [p * nlocp, ROWW], f32,
                               addr_space="Shared") for i in (1, 2)]
    er_loc = [nc.dram_tensor(f"er_loc{i}", [nlocp, ERW], f32) for i in (1, 2)]

    with tile.TileContext(nc) as tc:
        with tc.tile_pool(name="cst", bufs=1) as cst, \
             tc.tile_pool(name="big", bufs=1) as big, \
             tc.tile_pool(name="fe", bufs=2) as fep, \
             tc.tile_pool(name="xbp", bufs=3) as xbp, \
             tc.tile_pool(name="wk", bufs=6) as wk, \
             tc.tile_pool(name="ep", bufs=3) as ep, \
             tc.tile_pool(name="ps", bufs=1, space="PSUM") as ps:

            xT = cst.sbuf_tile_from(xT_in.ap())
            Wc1 = cst.sbuf_tile_from(W1_in.ap())
            Wc2 = cst.sbuf_tile_from(W2_in.ap())
            b2r = cst.sbuf_tile_from(b2r_in.ap())
            iota = cst.sbuf_tile_from(iota_in.ap())
            ident = cst.sbuf_tile_from(ident_in.ap())
            gA = cst.sbuf_tile_from(gA_in.ap())
            gB = cst.sbuf_tile_from(gB_in.ap())
            ger = cst.sbuf_tile_from(ger_in.ap())
            dstrow = cst.sbuf_tile_from(dstrow_in.ap())

            h_sb = big.tile([TILE, t_tiles * F], f32)
            hT_sb = big.tile([TILE, t_tiles * TILE], f32)

            def pre_phase(lhsT_sb, Wc, layer):
                """node-level matmuls -> featel_loc / er_loc, then AllGather."""
                for nt in range(t_tiles):
                    sl = slice(nt * TILE, (nt + 1) * TILE)
                    pf = ps.tile([TILE, F], f32, tag="pf", bufs=2, name=f"pf{layer}_{nt}")
                    nc.tensor.matmul(pf[:, :], lhsT=lhsT_sb[:, sl], rhs=Wc[:, 0:F],
                                     start=True, stop=True)
                    p8 = ps.tile([TILE, 2 * H], f32, tag="p8", bufs=1, name=f"p8{layer}_{nt}")
                    nc.tensor.matmul(p8[:, :], lhsT=lhsT_sb[:, sl], rhs=Wc[:, F:F + 2 * H],
                                     start=True, stop=True)
                    fel = ep.tile([TILE, ROWW], f32, tag="fel", name=f"fel{layer}_{nt}")
                    nc.vector.tensor_copy(fel[:, 0:F], pf[:, :])
                    nc.vector.tensor_copy(fel[:, F:F + H], p8[:, 0:H])
                    nc.vector.memset(fel[:, F + H:ROWW], 0.0)
                    ers = ep.tile([TILE, ERW], f32, tag="ers", name=f"ers{layer}_{nt}")
                    nc.vector.tensor_copy(ers[:, 0:H], p8[:, H:2 * H])
                    nc.vector.memset(ers[:, H:ERW], 0.0)
                    nc.sync.dma_start(fel_loc[layer][sl, :], fel[:, :])
                    nc.sync.dma_start(er_loc[layer][sl, :], ers[:, :])
                nc.gpsimd.collective_compute(
                    "AllGather", mybir.AluOpType.bypass, replica_groups=rg,
                    ins=[fel_loc[layer].ap().opt()], outs=[fel_full[layer].ap().opt()])

            fdt = mybir.dt.bfloat16 if BF16_AGG else f32

            def edge_phase(layer):
                """per-dst-tile gather + SDDMM + softmax-weighted aggregation."""
                for t in range(t_tiles):
                    ka, kb = KAs[t], KBs[t]
                    kt = ka + kb
                    o0, oa, ob = int(off[t]), int(offA[t]), int(offB[t])
                    fe = fep.tile([TILE, kt, ROWW], f32, tag="fe",
                                  padded_shape=[TILE, Kmax, ROWW], name=f"fe{layer}_{t}")
                    if ka:
                        nc.gpsimd.dma_gather(
                            fe[:, 0:ka, :], fel_full[layer].ap(),
                            gA[:, 8 * oa:8 * (oa + ka)],
                            ka * TILE, ka * TILE, ROWW, single_packet=False)
                    if kb:
                        nc.gpsimd.dma_gather(
                            fe[:, ka:kt, :], fel_full[layer].ap()[half:, :],
                            gB[:, 8 * ob:8 * (ob + kb)],
                            kb * TILE, kb * TILE, ROWW, single_packet=False)
                    ersb = fep.tile([TILE, kt, ERW], f32, tag="ersb",
                                    padded_shape=[TILE, Kmax, ERW], name=f"erb{layer}_{t}")
                    nc.gpsimd.dma_gather(
                        ersb[:, :, :], er_loc[layer].ap(),
                        ger[:, 8 * o0:8 * (o0 + kt)],
                        kt * TILE, kt * TILE, ERW, single_packet=False)
                    # batched SDDMM: logits -> lrelu -> exp(s) -> featw, whole tile
                    lg = wk.tile([TILE, kt * H], f32, tag="lg", bufs=3,
                                 padded_shape=[TILE, Kmax * H], name=f"lg{layer}_{t}")
                    nc.vector.tensor_tensor(lg[:, :], fe[:, :, F:F + H],
                                            ersb[:, :, 0:H], op=ALU.add)
                    lr = wk.tile([TILE, kt * H], f32, tag="lr", bufs=3,
                                 padded_shape=[TILE, Kmax * H], name=f"lr{layer}_{t}")
                    nc.vector.scalar_tensor_tensor(lr[:, :], lg[:, :], NEG, lg[:, :],
                                                   ALU.mult, ALU.max)
                    fw = wk.tile([TILE, kt, F + H], fdt, tag="fw", bufs=2,
                                 padded_shape=[TILE, Kmax, F + H], name=f"fw{layer}_{t}")
                    nc.scalar.activation(fw[:, :, F:F + H], lr[:, :], AF.Exp)
                    sv = fw[:, :, F:F + H]
                    s_b = bass.AP(sv.tensor, sv.offset,
                                  [sv.ap[0], [F + H, kt], [1, H], [0, HID]])
                    nc.vector.tensor_tensor(fw[:, :, 0:F], fe[:, :, 0:F], s_b,
                                            op=ALU.mult)
                    agg = ps.tile([TILE, F + H], f32, tag="agg", bufs=3,
                                  name=f"agg{layer}_{t}")
                    for k in range(kt):
                        col = o0 + k
                        O = wk.tile([TILE, TILE], fdt, tag="O", name=f"O{layer}_{t}_{k}")
                        nc.vector.tensor_scalar(O[:, :], iota[:, :],
                                                dstrow[:, col:col + 1], None,
                                                op0=ALU.is_equal)
                        nc.tensor.matmul(agg[:, :], lhsT=O[:, :], rhs=fw[:, k, :],
                                         start=(k == 0), stop=(k == kt - 1))
                    yield t, agg

            # ================= layer 1 =================
            pre_phase(xT, Wc1, 0)
            for t, agg in edge_phase(0):
                sl128 = slice(t * TILE, (t + 1) * TILE)
                slF = slice(t * F, (t + 1) * F)
                den = wk.tile([TILE, H], f32, tag="den", name=f"den1_{t}")
                nc.vector.tensor_scalar(den[:, :], agg[:, F:F + H], 1e-9, None, op0=ALU.max)
                rec = wk.tile([TILE, H], f32, tag="rec", name=f"rec1_{t}")
                nc.vector.reciprocal(rec[:, :], den[:, :])
                rst = ep.tile([TILE, F], f32, tag="rst", name=f"rst1_{t}")
                for h in range(H):
                    nc.scalar.activation(rst[:, h * HID:(h + 1) * HID],
                                         agg[:, h * HID:(h + 1) * HID],
                                         AF.Copy, scale=rec[:, h:h + 1])
                xb_t = xbp.tile([TILE, F], f32, tag="xb", name=f"xb_{t}")
                nc.sync.dma_start(xb_t[:, :], xb_in[:, slF])
                nc.vector.tensor_tensor(rst[:, :], rst[:, :], xb_t[:, :], op=ALU.add)
                # ELU -> h
                r1 = ep.tile([TILE, F], f32, tag="r1", name=f"r1_{t}")
                nc.scalar.activation(r1[:, :], rst[:, :], AF.Relu)
                r2 = ep.tile([TILE, F], f32, tag="r2", name=f"r2_{t}")
                nc.scalar.activation(r2[:, :], rst[:, :], AF.Relu, scale=-1.0)
                r3 = ep.tile([TILE, F], f32, tag="r3", name=f"r3_{t}")
                nc.scalar.activation(r3[:, :], r2[:, :], AF.Exp, scale=-1.0)
                nc.vector.scalar_tensor_tensor(h_sb[:, slF], r3[:, :], -1.0, r1[:, :],
                                               ALU.add, ALU.add)
                ptr = ps.tile([TILE, TILE], f32, tag="tr", bufs=2, name=f"tr_{t}")
                nc.tensor.transpose(ptr[:, :], h_sb[:, slF], ident[:, :])
                nc.vector.tensor_copy(hT_sb[:, sl128], ptr[:, :])

            # ================= layer 2 =================
            pre_phase(hT_sb, Wc2, 1)
            for t, agg in edge_phase(1):
                slF = slice(t * F, (t + 1) * F)
                den = wk.tile([TILE, H], f32, tag="den", name=f"den2_{t}")
                nc.vector.tensor_scalar(den[:, :], agg[:, F:F + H], 1e-9, None, op0=ALU.max)
                rec = wk.tile([TILE, H], f32, tag="rec", name=f"rec2_{t}")
                nc.vector.reciprocal(rec[:, :], den[:, :])
                rst = ep.tile([TILE, F], f32, tag="rst", name=f"rst2_{t}")
                for h in range(H):
                    nc.scalar.activation(rst[:, h * HID:(h + 1) * HID],
                                         agg[:, h * HID:(h + 1) * HID],
                                         AF.Copy, scale=rec[:, h:h + 1])
                nc.vector.tensor_tensor(rst[:, :], rst[:, :], h_sb[:, slF], op=ALU.add)
                nc.vector.tensor_tensor(rst[:, :], rst[:, :], b2r[:, :], op=ALU.add)
                m1 = ep.tile([TILE, OUTD], f32, tag="m1", name=f"m1_{t}")
                nc.vector.tensor_tensor(m1[:, :], rst[:, 0:OUTD], rst[:, OUTD:2 * OUTD],
                                        op=ALU.add)
                m2 = ep.tile([TILE, OUTD], f32, tag="m2", name=f"m2_{t}")
                nc.vector.tensor_tensor(m2[:, :], rst[:, 2 * OUTD:3 * OUTD],
                                        rst[:, 3 * OUTD:4 * OUTD], op=ALU.add)
                ot = ep.tile([TILE, OUTD], f32, tag="ot", name=f"ot_{t}")
                nc.vector.tensor_tensor(ot[:, :], m1[:, :], m2[:, :], op=ALU.add)
                of = ep.tile([TILE, OUTD], f32, tag="of", name=f"of_{t}")
                nc.vector.tensor_scalar(of[:, :], ot[:, :], 0.25, None, op0=ALU.mult)
                nc.sync.dma_start(out_ext[t * TILE:(t + 1) * TILE, :], of[:, :])

    nc.compile()
    return nc


# ----------------------------------------------------------------------------
# Host entry point
# ----------------------------------------------------------------------------

def make_inputs(x, W1, al1, ar1, b1, W2, al2, ar2, b2, src, dst, n=N, p=P):
    KAs, KBs, per_core = prep_edges(np.asarray(src), np.asarray(dst), n=n, p=p)
    Wcat1 = prep_weights(np.asarray(W1, np.float32), np.asarray(al1, np.float32),
                         np.asarray(ar1, np.float32))
    Wcat2 = prep_weights(np.asarray(W2, np.float32), np.asarray(al2, np.float32),
                         np.asarray(ar2, np.float32))
    node_in = prep_node_inputs(np.asarray(x, np.float32), np.asarray(b1, np.float32),
                               n=n, p=p)
    b2r = np.tile(np.asarray(b2, np.float32)[None, :], (TILE, 1))
    iota = np.tile(np.arange(TILE, dtype=np.float32), (TILE, 1))
    ident = np.eye(TILE, dtype=np.float32)
    in_maps = []
    for c in range(p):
        xT, xb = node_in[c]
        pc = per_core[c]
        gB = pc["gB"] if pc["gB"].shape[1] else np.zeros((TILE, 16), np.int16)
        in_maps.append(dict(
            xT=xT, xb=xb, Wcat1=Wcat1, Wcat2=Wcat2, b2r=b2r, iota=iota, ident=ident,
            gA=pc["gA"], gB=gB, ger=pc["ger"], dstrow=pc["dstrow"]))
    return KAs, KBs, in_maps


def kernel(x, W1, al1, ar1, b1, W2, al2, ar2, b2, src, dst, **run_kwargs):
    from concourse.bass_utils import run_bass_kernel_spmd
    KAs, KBs, in_maps = make_inputs(x, W1, al1, ar1, b1, W2, al2, ar2, b2, src, dst)
    nc = build_gat(KAs, KBs)
    res = run_bass_kernel_spmd(nc, in_maps, core_ids=list(range(P)), **run_kwargs)
    out = np.concatenate([r["out"][:NLOC] for r in res.results], axis=0)
    if run_kwargs:
        return out.astype(np.float32), res
    return out.astype(np.float32)



# revision 3
# speedup vs baseline: 3.1657x; 3.1657x over previous
"""GAT (2-layer, 4-head) distributed Bass kernel for Trainium2, 8 NeuronCores.

v2 design (vs baseline):
  - fp16 packed featel table: rows of 512 B ([feat(128 f16) | el(4 f16) | pad]),
    halving gather DMA volume vs fp32 768 B rows.
  - NO er dma_gather: er stays in SBUF per-core; the per-edge expansion
    er_e = er[dstrow_e] is a tiny PE matmul with a host-PREBUILT transposed
    one-hot OT (the graph is static, so O/OT are compile-time constants
    streamed from DRAM as contiguous fp16 slabs - no per-edge descriptors,
    no on-device one-hot builds).
  - Softmax-weighted aggregation agg = O^T @ [featw | s] also uses the
    prebuilt O slab (fp16 PE matmuls, 4x faster than fp32).
  - dma_gather calls round-robin over 4 SWDGE queues (Q7 prep runs in
    parallel across queues: measured 2.5x descriptor-prep throughput).
  - Single-pass softmax: alpha = exp(e)/sum(exp(e)); logits are O(1).

Per-core layout: core c owns nodes [c*NLOC, (c+1)*NLOC), padded to T*128.
Edges grouped by (dst owner, dst tile, src-half A/B) exactly as the baseline
(int16 gather indices limit the table to 32768 rows per gather base).
"""

import numpy as np

# ---- problem constants (hardcoded; kernel.py must be self-contained) ----
N = 50000
E = 800000
P = 8
IN = 128
HID = 32
H = 4
F = H * HID          # 128, same for both layers
OUTD = 32
NEG = 0.2
TILE = 128

NLOC = N // P        # 6250
T = (NLOC + TILE - 1) // TILE          # 49
NLOCP = T * TILE     # 6272

ROWE = 256           # featel table row elems (f16) -> 512 B
NQ = 4               # SWDGE queues


def _wrap16(idx):
    """[n] index list -> [128, n//16] int16, wrapped in 16 partitions and
    replicated across the 8 Q7 cores (dma_gather layout)."""
    a = np.asarray(idx).reshape(-1, 16).T
    return np.tile(a, (8, 1)).astype(np.int16)


# ----------------------------------------------------------------------------
# Host-side preprocessing
# ----------------------------------------------------------------------------

def prep_edges(src, dst, n=N, p=P):
    """Group edges by (dst owner, dst tile, src-half), pad each (core,tile,
    half) to common chunk counts KA_t/KB_t, and emit per-core index arrays
    plus prebuilt one-hot slabs.

    Returns (KAs, KBs, per_core): per_core[c] has
      gA   int16 [128, 8*sumKA]   wrapped featel-gather idxs, A half
      gB   int16 [128, 8*sumKB]   wrapped featel-gather idxs, B half (rebased)
      Otab  f16  [128, sumK*128]  per-chunk one-hot O[e, r] (pad rows: 0)
      OTtab f16  [128, sumK*128]  per-chunk transposed one-hot OT[r, e]
    """
    nloc = n // p
    t_tiles = (nloc + TILE - 1) // TILE
    nlocp = t_tiles * TILE
    half = (p // 2) * nlocp

    owner = dst // nloc
    loc = dst - owner * nloc
    tl = loc // TILE
    row = loc - tl * TILE

    sowner = src // nloc
    pgid = sowner * nlocp + (src - sowner * nloc)
    hb = (pgid >= half).astype(np.int64)          # 0 = A, 1 = B

    order = np.lexsort((hb, tl, owner))
    owner_s = owner[order]
    tl_s = tl[order]
    hb_s = hb[order]
    row_s = row[order].astype(np.int64)
    pgid_s = pgid[order].astype(np.int64)

    counts = np.zeros((p, t_tiles, 2), dtype=np.int64)
    np.add.at(counts, (owner_s, tl_s, hb_s), 1)
    KAs = (-(-counts[:, :, 0] // TILE)).max(axis=0)
    KBs = (-(-counts[:, :, 1] // TILE)).max(axis=0)
    KAs = np.maximum(KAs, (KAs + KBs) == 0)       # ensure >=1 chunk per tile
    Ks = KAs + KBs
    off = np.concatenate([[0], np.cumsum(Ks)]).astype(int)
    offA = np.concatenate([[0], np.cumsum(KAs)]).astype(int)
    offB = np.concatenate([[0], np.cumsum(KBs)]).astype(int)
    sumK, sumKA, sumKB = int(off[-1]), int(offA[-1]), int(offB[-1])

    grp = (owner_s * t_tiles + tl_s) * 2 + hb_s
    gcnt = np.bincount(grp, minlength=p * t_tiles * 2)
    gstart = np.concatenate([[0], np.cumsum(gcnt)])
    within = np.arange(len(src)) - gstart[grp]
    k = within // TILE
    prt = within - k * TILE
    # chunk column in the full per-tile layout (A chunks first, then B)
    col = off[tl_s] + np.where(hb_s == 0, k, KAs[tl_s] + k)

    per_core = []
    for c in range(p):
        m = owner_s == c
        dstrow = np.full((TILE, sumK), -1, dtype=np.int64)
        dstrow[prt[m], col[m]] = row_s[m]
        gfull = np.zeros((TILE, sumK), dtype=np.int64)
        gfull[prt[m], col[m]] = pgid_s[m]

        # prebuilt one-hots: O[e, r] = (dstrow[e]==r), pad slots (-1) all-zero
        eye = np.concatenate([np.eye(TILE, dtype=np.float16),
                              np.zeros((1, TILE), np.float16)])  # row -1 -> 0
        Otab = np.empty((TILE, sumK * TILE), dtype=np.float16)
        OTtab = np.empty((TILE, sumK * TILE), dtype=np.float16)
        for q in range(sumK):
            Oq = eye[dstrow[:, q]]                # [128e, 128r]
            Otab[:, q * TILE:(q + 1) * TILE] = Oq
            OTtab[:, q * TILE:(q + 1) * TILE] = Oq.T

        # flatten chunk cols into wrapped idx streams
        gA = np.zeros((TILE, 8 * sumKA), dtype=np.int16)
        gB = np.zeros((TILE, 8 * sumKB), dtype=np.int16)
        for t in range(t_tiles):
            ka, kb = int(KAs[t]), int(KBs[t])
            o, oa, ob = off[t], offA[t], offB[t]
            if ka:
                ia = gfull[:, o:o + ka].T.reshape(-1)          # i = k*128+p
                gA[:, 8 * oa:8 * (oa + ka)] = _wrap16(ia)
            if kb:
                ib = gfull[:, o + ka:o + ka + kb].T.reshape(-1) - half
                ib[ib < 0] = 0                                 # pad slots
                gB[:, 8 * ob:8 * (ob + kb)] = _wrap16(ib)
        per_core.append(dict(gA=gA, gB=gB, Otab=Otab, OTtab=OTtab))
    return [int(x) for x in KAs], [int(x) for x in KBs], per_core


def prep_weights(W, al, ar):
    """[W | W@al per head | W@ar per head] -> [in, F+2H] float16."""
    Wr = W.reshape(W.shape[0], H, -1)
    wal = np.einsum('ihd,hd->ih', Wr, al)
    war = np.einsum('ihd,hd->ih', Wr, ar)
    return np.concatenate([W, wal, war], axis=1).astype(np.float16)


def prep_node_inputs(x, b1, n=N, p=P):
    """Per-core xT ([IN, NLOCP] f16, lhsT layout) and xb ([128, T*IN] f32,
    tile-row-major residual layout, bias prefolded)."""
    nloc = n // p
    t_tiles = (nloc + TILE - 1) // TILE
    nlocp = t_tiles * TILE
    outs = []
    for c in range(p):
        xl = np.zeros((nlocp, x.shape[1]), dtype=np.float32)
        xl[:nloc] = x[c * nloc:(c + 1) * nloc]
        xT = np.ascontiguousarray(xl.T).astype(np.float16)
        xb = (xl + b1[None, :]).reshape(t_tiles, TILE, -1).transpose(1, 0, 2)
        xb = np.ascontiguousarray(xb.reshape(TILE, -1))
        outs.append((xT, xb))
    return outs


# ----------------------------------------------------------------------------
# Bass IR builder
# ----------------------------------------------------------------------------

def build_gat(KAs, KBs, n=N, p=P, in_dim=IN):
    import concourse.bass as bass
    import concourse.bacc as bacc
    import concourse.mybir as mybir
    import concourse.tile as tile

    f32 = mybir.dt.float32
    f16 = mybir.dt.float16
    i16 = mybir.dt.int16
    AF = mybir.ActivationFunctionType
    ALU = mybir.AluOpType

    nloc = n // p
    t_tiles = (nloc + TILE - 1) // TILE
    nlocp = t_tiles * TILE
    half = (p // 2) * nlocp
    KAs = list(KAs)
    KBs = list(KBs)
    Ks = [a + b for a, b in zip(KAs, KBs)]
    off = np.concatenate([[0], np.cumsum(Ks)]).astype(int)
    offA = np.concatenate([[0], np.cumsum(KAs)]).astype(int)
    offB = np.concatenate([[0], np.cumsum(KBs)]).astype(int)
    sumK, sumKA, sumKB = int(off[-1]), int(offA[-1]), int(offB[-1])
    Kmax = max(Ks)
    rg = [list(range(p))]

    nc = bacc.Bacc("TRN2", target_bir_lowering=False, num_swdge_queues=NQ)

    # ---- I/O ----
    xT_in = nc.dram_tensor("xT", [in_dim, nlocp], f16, kind="ExternalInput")
    xb_in = nc.dram_tensor("xb", [TILE, t_tiles * in_dim], f32, kind="ExternalInput")
    W1_in = nc.dram_tensor("Wcat1", [in_dim, F + 2 * H], f16, kind="ExternalInput")
    W2_in = nc.dram_tensor("Wcat2", [F, F + 2 * H], f16, kind="ExternalInput")
    b2r_in = nc.dram_tensor("b2r", [TILE, F], f32, kind="ExternalInput")
    ident_in = nc.dram_tensor("ident", [TILE, TILE], f16, kind="ExternalInput")
    gA_in = nc.dram_tensor("gA", [TILE, 8 * sumKA], i16, kind="ExternalInput")
    gB_in = nc.dram_tensor("gB", [TILE, max(8 * sumKB, 16)], i16, kind="ExternalInput")
    Otab_in = nc.dram_tensor("Otab", [TILE, sumK * TILE], f16, kind="ExternalInput")
    OTtab_in = nc.dram_tensor("OTtab", [TILE, sumK * TILE], f16, kind="ExternalInput")
    out_ext = nc.dram_tensor("out", [nlocp, OUTD], f32, kind="ExternalOutput")

    # ---- internal DRAM ----
    fel_loc = [nc.dram_tensor(f"fel_loc{i}", [nlocp, ROWE], f16) for i in (1, 2)]
    fel_full = [nc.dram_tensor(f"fel_full{i}", [p * nlocp, ROWE], f16,
                               addr_space="Shared") for i in (1, 2)]

    qrr = [0]  # SWDGE queue round-robin

    with tile.TileContext(nc) as tc:
        with tc.tile_pool(name="cst", bufs=1) as cst, \
             tc.tile_pool(name="big", bufs=1) as big, \
             tc.tile_pool(name="fe", bufs=2) as fep, \
             tc.tile_pool(name="osl", bufs=2) as osl, \
             tc.tile_pool(name="xbp", bufs=3) as xbp, \
             tc.tile_pool(name="wk", bufs=6) as wk, \
             tc.tile_pool(name="ep", bufs=3) as ep, \
             tc.tile_pool(name="ps", bufs=1, space="PSUM") as ps:

            xT = cst.sbuf_tile_from(xT_in.ap())
            Wc1 = cst.sbuf_tile_from(W1_in.ap())
            Wc2 = cst.sbuf_tile_from(W2_in.ap())
            b2r = cst.sbuf_tile_from(b2r_in.ap())
            ident = cst.sbuf_tile_from(ident_in.ap())
            gA = cst.sbuf_tile_from(gA_in.ap())
            gB = cst.sbuf_tile_from(gB_in.ap())

            h_sb = big.tile([TILE, t_tiles * F], f32)
            h16_sb = big.tile([TILE, t_tiles * F], f16)
            hT_sb = big.tile([TILE, t_tiles * TILE], f16)
            er_sb = [big.tile([TILE, t_tiles * H], f16, name=f"er{i}")
                     for i in (0, 1)]

            def pre_phase(lhsT_sb, Wc, layer):
                """node-level matmuls -> featel_loc (f16) + er_sb, AllGather."""
                for nt in range(t_tiles):
                    sl = slice(nt * TILE, (nt + 1) * TILE)
                    pf = ps.tile([TILE, F + 2 * H], f32, tag="pf", bufs=2,
                                 name=f"pf{layer}_{nt}")
                    nc.tensor.matmul(pf[:, :], lhsT=lhsT_sb[:, sl],
                                     rhs=Wc[:, :], start=True, stop=True)
                    fel = ep.tile([TILE, ROWE], f16, tag="fel", name=f"fel{layer}_{nt}")
                    nc.vector.tensor_copy(fel[:, 0:F + H], pf[:, 0:F + H])
                    nc.vector.tensor_copy(er_sb[layer][:, nt * H:(nt + 1) * H],
                                          pf[:, F + H:F + 2 * H])
                    nc.sync.dma_start(fel_loc[layer][sl, :], fel[:, :])
                nc.gpsimd.collective_compute(
                    "AllGather", mybir.AluOpType.bypass, replica_groups=rg,
                    ins=[fel_loc[layer].ap().opt()], outs=[fel_full[layer].ap().opt()])

            def edge_phase(layer):
                """per-dst-tile gather + SDDMM + softmax-weighted aggregation."""
                for t in range(t_tiles):
                    ka, kb = KAs[t], KBs[t]
                    kt = ka + kb
                    o0, oa, ob = int(off[t]), int(offA[t]), int(offB[t])
                    fe = fep.tile([TILE, kt, ROWE], f16, tag="fe",
                                  padded_shape=[TILE, Kmax, ROWE], name=f"fe{layer}_{t}")
                    for g0 in range(0, ka, 8):
                        gk = min(8, ka - g0)
                        nc.gpsimd.dma_gather(
                            fe[:, g0:g0 + gk, :], fel_full[layer].ap(),
                            gA[:, 8 * (oa + g0):8 * (oa + g0 + gk)],
                            gk * TILE, gk * TILE, ROWE, single_packet=False,
                            queue_num=qrr[0] % NQ)
                        qrr[0] += 1
                    for g0 in range(0, kb, 8):
                        gk = min(8, kb - g0)
                        nc.gpsimd.dma_gather(
                            fe[:, ka + g0:ka + g0 + gk, :],
                            fel_full[layer].ap()[half:, :],
                            gB[:, 8 * (ob + g0):8 * (ob + g0 + gk)],
                            gk * TILE, gk * TILE, ROWE, single_packet=False,
                            queue_num=qrr[0] % NQ)
                        qrr[0] += 1
                    # O / OT slabs (prebuilt one-hots, contiguous stream)
                    osb = osl.tile([TILE, kt, TILE], f16, tag="osb",
                                   padded_shape=[TILE, Kmax, TILE], name=f"os{layer}_{t}")
                    nc.sync.dma_start(osb[:, :, :],
                                      Otab_in[:, o0 * TILE:(o0 + kt) * TILE])
                    otsb = osl.tile([TILE, kt, TILE], f16, tag="otsb",
                                    padded_shape=[TILE, Kmax, TILE], name=f"ot{layer}_{t}")
                    nc.sync.dma_start(otsb[:, :, :],
                                      OTtab_in[:, o0 * TILE:(o0 + kt) * TILE])
                    # er expansion: er_strip[:, 4k:4k+4] = OT_k^T @ er_tile
                    ers = ps.tile([TILE, kt * H], f32, tag="ers", bufs=2,
                                  padded_shape=[TILE, Kmax * H], name=f"ers{layer}_{t}")
                    for k in range(kt):
                        nc.tensor.matmul(ers[:, k * H:(k + 1) * H],
                                         lhsT=otsb[:, k, :],
                                         rhs=er_sb[layer][:, t * H:(t + 1) * H],
                                         start=True, stop=True)
                    # batched SDDMM: logits -> lrelu -> exp(s) -> featw
                    lg = wk.tile([TILE, kt * H], f32, tag="lg", bufs=3,
                                 padded_shape=[TILE, Kmax * H], name=f"lg{layer}_{t}")
                    nc.vector.tensor_tensor(lg[:, :], fe[:, :, F:F + H],
                                            ers[:, :], op=ALU.add)
                    lr = wk.tile([TILE, kt * H], f32, tag="lr", bufs=3,
                                 padded_shape=[TILE, Kmax * H], name=f"lr{layer}_{t}")
                    nc.vector.scalar_tensor_tensor(lr[:, :], lg[:, :], NEG, lg[:, :],
                                                   ALU.mult, ALU.max)
                    fw = wk.tile([TILE, kt, F + H], f16, tag="fw", bufs=2,
                                 padded_shape=[TILE, Kmax, F + H], name=f"fw{layer}_{t}")
                    nc.scalar.activation(fw[:, :, F:F + H], lr[:, :], AF.Exp)
                    sv = fw[:, :, F:F + H]
                    s_b = bass.AP(sv.tensor, sv.offset,
                                  [sv.ap[0], [F + H, kt], [1, H], [0, HID]])
                    nc.vector.tensor_tensor(fw[:, :, 0:F], fe[:, :, 0:F], s_b,
                                            op=ALU.mult)
                    agg = ps.tile([TILE, F + H], f32, tag="agg", bufs=2,
                                  name=f"agg{layer}_{t}")
                    for k in range(kt):
                        nc.tensor.matmul(agg[:, :], lhsT=osb[:, k, :],
                                         rhs=fw[:, k, :],
                                         start=(k == 0), stop=(k == kt - 1))
                    yield t, agg

            # ================= layer 1 =================
            pre_phase(xT, Wc1, 0)
            for t, agg in edge_phase(0):
                sl128 = slice(t * TILE, (t + 1) * TILE)
                slF = slice(t * F, (t + 1) * F)
                den = wk.tile([TILE, H], f32, tag="den", name=f"den1_{t}")
                nc.vector.tensor_scalar(den[:, :], agg[:, F:F + H], 1e-9, None, op0=ALU.max)
                rec = wk.tile([TILE, H], f32, tag="rec", name=f"rec1_{t}")
                nc.vector.reciprocal(rec[:, :], den[:, :])
                rst = ep.tile([TILE, F], f32, tag="rst", name=f"rst1_{t}")
                for h in range(H):
                    nc.scalar.activation(rst[:, h * HID:(h + 1) * HID],
                                         agg[:, h * HID:(h + 1) * HID],
                                         AF.Copy, scale=rec[:, h:h + 1])
                xb_t = xbp.tile([TILE, F], f32, tag="xb", name=f"xb_{t}")
                nc.sync.dma_start(xb_t[:, :], xb_in[:, slF])
                nc.vector.tensor_tensor(rst[:, :], rst[:, :], xb_t[:, :], op=ALU.add)
                # ELU -> h
                r1 = ep.tile([TILE, F], f32, tag="r1", name=f"r1_{t}")
                nc.scalar.activation(r1[:, :], rst[:, :], AF.Relu)
                r2 = ep.tile([TILE, F], f32, tag="r2", name=f"r2_{t}")
                nc.scalar.activation(r2[:, :], rst[:, :], AF.Relu, scale=-1.0)
                r3 = ep.tile([TILE, F], f32, tag="r3", name=f"r3_{t}")
                nc.scalar.activation(r3[:, :], r2[:, :], AF.Exp, scale=-1.0)
                nc.vector.scalar_tensor_tensor(h_sb[:, slF], r3[:, :], -1.0, r1[:, :],
                                               ALU.add, ALU.add)
                nc.vector.tensor_copy(h16_sb[:, slF], h_sb[:, slF])
                ptr = ps.tile([TILE, TILE], f16, tag="tr", bufs=1, name=f"tr_{t}")
                nc.tensor.transpose(ptr[:, :], h16_sb[:, slF], ident[:, :])
                nc.vector.tensor_copy(hT_sb[:, sl128], ptr[:, :])

            # ================= layer 2 =================
            pre_phase(hT_sb, Wc2, 1)
            for t, agg in edge_phase(1):
                slF = slice(t * F, (t + 1) * F)
                den = wk.tile([TILE, H], f32, tag="den", name=f"den2_{t}")
                nc.vector.tensor_scalar(den[:, :], agg[:, F:F + H], 1e-9, None, op0=ALU.max)
                rec = wk.tile([TILE, H], f32, tag="rec", name=f"rec2_{t}")
                nc.vector.reciprocal(rec[:, :], den[:, :])
                rst = ep.tile([TILE, F], f32, tag="rst", name=f"rst2_{t}")
                for h in range(H):
                    nc.scalar.activation(rst[:, h * HID:(h + 1) * HID],
                                         agg[:, h * HID:(h + 1) * HID],
                                         AF.Copy, scale=rec[:, h:h + 1])
                nc.vector.tensor_tensor(rst[:, :], rst[:, :], h_sb[:, slF], op=ALU.add)
                nc.vector.tensor_tensor(rst[:, :], rst[:, :], b2r[:, :], op=ALU.add)
                m1 = ep.tile([TILE, OUTD], f32, tag="m1", name=f"m1_{t}")
                nc.vector.tensor_tensor(m1[:, :], rst[:, 0:OUTD], rst[:, OUTD:2 * OUTD],
                                        op=ALU.add)
                m2 = ep.tile([TILE, OUTD], f32, tag="m2", name=f"m2_{t}")
                nc.vector.tensor_tensor(m2[:, :], rst[:, 2 * OUTD:3 * OUTD],
                                        rst[:, 3 * OUTD:4 * OUTD], op=ALU.add)
                ot = ep.tile([TILE, OUTD], f32, tag="ot", name=f"ot_{t}")
                nc.vector.tensor_tensor(ot[:, :], m1[:, :], m2[:, :], op=ALU.add)
                of = ep.tile([TILE, OUTD], f32, tag="of", name=f"of_{t}")
                nc.vector.tensor_scalar(of[:, :], ot[:, :], 0.25, None, op0=ALU.mult)
                nc.sync.dma_start(out_ext[t * TILE:(t + 1) * TILE, :], of[:, :])

    nc.compile()
    return nc


# ----------------------------------------------------------------------------
# Host entry point
# ----------------------------------------------------------------------------

def make_inputs(x, W1, al1, ar1, b1, W2, al2, ar2, b2, src, dst, n=N, p=P):
    KAs, KBs, per_core = prep_edges(np.asarray(src), np.asarray(dst), n=n, p=p)
    Wcat1 = prep_weights(np.asarray(W1, np.float32), np.asarray(al1, np.float32),
                         np.asarray(ar1, np.float32))
    Wcat2 = prep_weights(np.asarray(W2, np.float32), np.asarray(al2, np.float32),
                         np.asarray(ar2, np.float32))
    node_in = prep_node_inputs(np.asarray(x, np.float32), np.asarray(b1, np.float32),
                               n=n, p=p)
    b2r = np.tile(np.asarray(b2, np.float32)[None, :], (TILE, 1))
    ident = np.eye(TILE, dtype=np.float16)
    in_maps = []
    for c in range(p):
        xT, xb = node_in[c]
        pc = per_core[c]
        gB = pc["gB"] if pc["gB"].shape[1] else np.zeros((TILE, 16), np.int16)
        in_maps.append(dict(
            xT=xT, xb=xb, Wcat1=Wcat1, Wcat2=Wcat2, b2r=b2r, ident=ident,
            gA=pc["gA"], gB=gB, Otab=pc["Otab"], OTtab=pc["OTtab"]))
    return KAs, KBs, in_maps


def kernel(x, W1, al1, ar1, b1, W2, al2, ar2, b2, src, dst, **run_kwargs):
    from concourse.bass_utils import run_bass_kernel_spmd
    KAs, KBs, in_maps = make_inputs(x, W1, al1, ar1, b1, W2, al2, ar2, b2, src, dst)
    nc = build_gat(KAs, KBs)
    res = run_bass_kernel_spmd(nc, in_maps, core_ids=list(range(P)), **run_kwargs)
    out = np.concatenate([r["out"][:NLOC] for r in res.results], axis=0)
    if run_kwargs:
        return out.astype(np.float32), res
    return out.astype(np.float32)
